# revision 20
# baseline (speedup 1.0000x reference)
"""Trainium2 Bass kernel for nn_ExpertDistillationLoss.

Strategy (data-parallel over batch, 8 cores, 1 batch element each):
  - Device (per core): the FLOP-heavy expert-MSE pipeline.
      d.T[h, s] = W_s.sh.T - W_t.th.T computed as one concatenated fp8
      DoubleRow GEMM (W stationary & SBUF-resident, loaded once; host
      pre-transposed layouts; f32 PSUM accumulation).
      mean_base via ACT square + per-tile ones-matmul PSUM accumulation.
      cross+quad terms fused into one PSUM accumulator V[s, 256] built from
      (a) fp8 DoubleRow P-matmuls of dT m-tile PAIRS against host-prescaled
          B_cat and
      (b) one fp8 DoubleRow Gram matmul per token tile (as/at paired),
      then a broadcasted DVE multiply/reduce against a_s/a_t.
      Device output per core: feat partial = sum wsel*mse (1 scalar).
  - Host: input sharding/layout, the K=3 MC sampling scan (gates-only, exact
    argmax semantics), method-B losses, and the final scalar combine.
"""

import numpy as np
import ml_dtypes

B, S, H, E, R, K = 8, 2048, 2048, 8, 16, 3
ALPHA = 0.5
LAMBDA_COV = 0.5
BETA_ENT = 0.1
TEMP_LO, TEMP_HI = 0.5, 1.5
SCALE_T = 2.0
SCALE_S = 2.0
EPS = 1e-8

NM = 16                # output h-tiles (128 rows each)
NKX = 32               # k-tiles: 16 student + 16 teacher
NC4 = 4                # 512-token chunks
NSUB = 4               # 128-token subchunks per chunk
NCH = 16               # 128-token chunks over S

BF16 = ml_dtypes.bfloat16
FP8 = ml_dtypes.float8_e4m3fn
WSCALE = 64.0          # W pre-scale so fp8 e4m3 stays in normal range
DCOPY = 0.25           # dT = DCOPY * pd = (WSCALE*DCOPY) * d = 16 d
ALPHA_V = 131072.0     # 2**17: common scale carried by the V accumulator
BC_F = ALPHA_V * 2.0 * SCALE_S / (H * WSCALE * DCOPY)   # = 16.0
GC_F = ALPHA_V * SCALE_S * SCALE_T / H                  # = 256.0

_PROGRAM_CACHE = {}


# ----------------------------------------------------------------------------
# device program
# ----------------------------------------------------------------------------

def _build_program(db_nonzero: bool, debug_out: bool = False):
    import concourse.bacc as bacc
    import concourse.tile as tile
    from concourse import mybir

    f32 = mybir.dt.float32
    fp8 = mybir.dt.float8e4
    DR = mybir.MatmulPerfMode.DoubleRow
    ALU = mybir.AluOpType
    AX = mybir.AxisListType

    KT = NKX + (1 if db_nonzero else 0)   # extra k-tile carries the bias row
    WB = KT * 128                          # W cols per m-tile
    XB = KT * 512                          # x cols per 512-token chunk

    nc = bacc.Bacc("TRN2", target_bir_lowering=False, debug=False)

    # DRAM inputs (per-core shapes; layouts are host-prepared)
    d_xc = nc.dram_tensor("xc", [128, NC4, XB], fp8, kind="ExternalInput").ap()
    d_Wc = nc.dram_tensor("Wc", [128, NM, WB], fp8, kind="ExternalInput").ap()
    d_Bc = nc.dram_tensor("Bc", [128, 8 * 512], fp8, kind="ExternalInput").ap()
    d_Gc = nc.dram_tensor("Gc", [16, 512], fp8, kind="ExternalInput").ap()
    d_aT = nc.dram_tensor("aT", [16, 2 * S], fp8, kind="ExternalInput").ap()
    d_ac = nc.dram_tensor("acat", [128, NCH * 32], f32, kind="ExternalInput").ap()
    d_wsel = nc.dram_tensor("wsel", [128, 128], f32, kind="ExternalInput").ap()
    d_wsele = nc.dram_tensor("wsel_e", [128, 16], f32, kind="ExternalInput").ap()
    d_onesH = nc.dram_tensor("onesH", [128, 1], f32, kind="ExternalInput").ap()

    d_feat = nc.dram_tensor("feat", [128, 1], f32, kind="ExternalOutput").ap()
    if debug_out:
        d_msed = nc.dram_tensor("mse_dbg", [128, 128], f32, kind="ExternalOutput").ap()
        d_mbd = nc.dram_tensor("mb_dbg", [128, 16], f32, kind="ExternalOutput").ap()

    with tile.TileContext(nc) as tc:
        with (
            tc.tile_pool(name="const", bufs=1) as cp,
            tc.tile_pool(name="dT", bufs=2) as dp,
            tc.tile_pool(name="sq", bufs=3) as qp,
            tc.tile_pool(name="vc", bufs=4) as vp,
            tc.tile_pool(name="pd", bufs=2, space="PSUM") as pd,
            tc.tile_pool(name="pv", bufs=4, space="PSUM") as pv,
            tc.tile_pool(name="pm", bufs=2, space="PSUM") as pm,
        ):
            # ---- SBUF tiles ----
            Gc_sb = cp.tile([16, 512], fp8, tag="Gc")
            aT_sb = cp.tile([16, 2 * S], fp8, tag="aT")
            Wc = cp.tile([128, NM * WB], fp8, tag="Wc")
            xc = cp.tile([128, NC4 * XB], fp8, tag="xc")
            Bc = cp.tile([128, 8 * 512], fp8, tag="Bc")
            acat_sb = cp.tile([128, NCH * 32], f32, tag="acat")
            wsel = cp.tile([128, 128], f32, tag="wsel")
            wsele = cp.tile([128, 16], f32, tag="wsele")
            onesH = cp.tile([128, 1], f32, tag="onesH")
            mse_sb = cp.tile([128, 128], f32, tag="mse")
            mb_sb = cp.tile([128, 16], f32, tag="mb")
            facc = cp.tile([128, 1], f32, tag="facc")
            nc.vector.memset(facc[:], 0.0)

            # ---- DMA emission order (HWDGE serializes at ~625ns/DMA and the
            # DMA bus at ~360B/ns shared; order = need order on the PE).
            # W must stream ahead of the PE's ~1.9us/m-tile cadence, so after
            # chunk 0's x data the W tiles go out back-to-back; later x chunks
            # ride behind the full W set.
            dma = nc.sync.dma_start
            dma(Gc_sb[:], d_Gc)
            dma(aT_sb[:], d_aT)
            dma(Wc[:, 0:WB], d_Wc[:, 0, :])
            dma(xc[:, 0:XB], d_xc[:, 0, :])               # chunk 0 in one go
            dma(Wc[:, WB:2 * WB], d_Wc[:, 1, :])
            dma(Bc[:], d_Bc)
            dma(onesH[:], d_onesH)
            for m in range(2, 16):
                dma(Wc[:, m * WB:(m + 1) * WB], d_Wc[:, m, :])
            for q in range(4):                             # c1 in 4 pieces
                dma(xc[:, XB + q * 4096:XB + (q + 1) * 4096],
                    d_xc[:, 1, q * 4096:(q + 1) * 4096])
            dma(acat_sb[:], d_ac)
            dma(wsel[:], d_wsel)
            dma(wsele[:], d_wsele)
            dma(xc[:, 2 * XB:3 * XB], d_xc[:, 2, :])
            dma(xc[:, 3 * XB:4 * XB], d_xc[:, 3, :])

            # ---- views ----
            W4 = Wc[:].rearrange("p (m k c) -> p m k c", m=NM, k=KT)
            x4 = xc[:].rearrange("p (n k s) -> p n k s", n=NC4, k=KT)
            aT2 = aT_sb[:].rearrange("p (j s) -> p j s", j=2)
            Gc2 = Gc_sb[:].rearrange("p (j g) -> p j g", j=2)
            Bc3 = Bc[:].rearrange("p (mp j g) -> p mp j g", mp=8, j=2)

            # PSUM rule (probe-verified): a bank holds ONE open accumulation
            # group; a start=True wipes any other OPEN group's partials in
            # that bank (committed/stopped results survive). So: V banks run
            # one sub's full chain at a time (subs 0/2 during the m-loop,
            # subs 1/3 afterwards from the dT cache), and mean_base uses
            # per-(m,sub) single-shot matmuls + a DVE reduction over m.
            V_of = {}    # c -> [2 psum tiles of [128, 512] (2 subs each)]
            mb_of = {}   # c -> [128, 64] psum tile of per-(m,sub) sums
            sq_of = {}   # (c, m) -> sq tile
            dT_of = {}   # c -> [128, 8*1024] fp8 dT cache (mp, j, 512)

            def Vap(c, sub):
                t = V_of[c][sub // 2]
                return t[:, (sub % 2) * 256:(sub % 2) * 256 + 256]

            def emit_u(c, subs):
                for sub in subs:
                    t0 = c * 512 + sub * 128
                    nc.tensor.matmul(Vap(c, sub), aT2[:, :, t0:t0 + 128],
                                     Gc2, start=True, stop=False,
                                     perf_mode=DR)

            def emit_pmm(c, mp, subs):
                dT3 = dT_of[c][:].rearrange("p (mp j s) -> p mp j s",
                                            mp=8, j=2)
                for sub in subs:
                    nc.tensor.matmul(
                        Vap(c, sub),
                        dT3[:, mp, :, sub * 128:sub * 128 + 128],
                        Bc3[:, mp], start=False, stop=(mp == 7),
                        perf_mode=DR)

            def emit_start(c):
                V_of[c] = [pv.tile([128, 512], f32, tag="V", name=f"V_{c}_{i}")
                           for i in range(2)]
                mb_of[c] = pm.tile([128, 64], f32, tag="mbp", name=f"mb_{c}")
                dT_of[c] = dp.tile([128, 8 * 1024], fp8, tag="dT",
                                   name=f"dTall_{c}")
                emit_u(c, (0, 2))

            def emit_kloop(c, m):
                pdt = pd.tile([128, 512], f32, tag="pd", name=f"pd_{c}_{m}")
                for kp in range(NKX // 2):
                    nc.tensor.matmul(
                        pdt[:], W4[:, m, 2 * kp:2 * kp + 2, :],
                        x4[:, c, 2 * kp:2 * kp + 2, :],
                        start=(kp == 0),
                        stop=(kp == NKX // 2 - 1 and KT == NKX),
                        perf_mode=DR)
                if KT > NKX:
                    # bias tail tile: plain (non-DoubleRow) fp8 matmul
                    nc.tensor.matmul(pdt[:], W4[:, m, NKX:NKX + 1, :],
                                     x4[:, c, NKX:NKX + 1, :],
                                     start=False, stop=True)
                # ACT ops run async while PE streams the next k-loop
                nc.scalar.mul(dT_of[c][:, m * 512:m * 512 + 512],
                              pdt[:], DCOPY)
                sq = qp.tile([128, 512], f32, tag="sq", name=f"sq_{c}_{m}")
                nc.scalar.square(sq[:], pdt[:])
                sq_of[(c, m)] = sq

            def emit_leftover(c, m):
                # P-matmuls first: they gate the consume/feat tail chain,
                # and the dT copy lands on ACT before the square does
                if m % 2 == 1:
                    emit_pmm(c, m // 2, (0, 2))
                # mean_base: per-(m,sub) single-shot ones-matmuls
                sq = sq_of.pop((c, m))
                mbp = mb_of[c]
                for sub in range(NSUB):
                    col = m * NSUB + sub
                    nc.tensor.matmul(mbp[:, col:col + 1],
                                     sq[:, sub * 128:sub * 128 + 128],
                                     onesH[:], start=True, stop=True)

            def emit_oddsubs(c):
                # subs 1/3 full chains after subs 0/2 committed (stop at mp7)
                emit_u(c, (1, 3))
                for mp in range(8):
                    emit_pmm(c, mp, (1, 3))

            def emit_consume(c, subs):
                for sub in subs:
                    ch = c * NSUB + sub
                    ab = acat_sb[:, ch * 32:(ch + 1) * 32].rearrange(
                        "p (t r) -> p t r", t=2)
                    ab = ab.unsqueeze(2).broadcast_to([128, 2, 8, 16])
                    va = Vap(c, sub)
                    prod = vp.tile([128, 256], f32, tag="prod",
                                   name=f"prod_{ch}")
                    nc.vector.tensor_tensor(
                        prod[:].rearrange("p (t e r) -> p t e r", t=2, e=8),
                        va.rearrange("p (t e r) -> p t e r", t=2, e=8),
                        ab, ALU.mult)
                    red = vp.tile([128, 16], f32, tag="red", name=f"red_{ch}")
                    nc.vector.tensor_reduce(
                        red[:],
                        prod[:].rearrange("p (t e r) -> p t e r", t=2, e=8),
                        axis=AX.X, op=ALU.add)
                    nc.vector.tensor_add(mse_sb[:, ch * 8:(ch + 1) * 8],
                                         red[:, 0:8], red[:, 8:16])

            def emit_feat(c):
                # fold this chunk's mse and mean_base into the running facc
                V_of.pop(c)
                dT_of.pop(c)
                mbp = mb_of.pop(c)
                nc.vector.tensor_reduce(
                    mb_sb[:, c * NSUB:(c + 1) * NSUB],
                    mbp[:].rearrange("p (m s) -> p s m", m=NM),
                    axis=AX.X, op=ALU.add)
                scr = vp.tile([128, 32], f32, tag="scr", name=f"scr_{c}")
                nc.vector.tensor_mul(scr[:], mse_sb[:, c * 32:(c + 1) * 32],
                                     wsel[:, c * 32:(c + 1) * 32])
                red = vp.tile([128, 1], f32, tag="fred", name=f"fred_{c}")
                nc.vector.tensor_reduce(red[:], scr[:], axis=AX.X, op=ALU.add)
                nc.vector.tensor_add(facc[:], facc[:], red[:])
                scr2 = vp.tile([128, 4], f32, tag="scr2", name=f"scr2_{c}")
                nc.vector.tensor_mul(scr2[:], mb_sb[:, c * 4:(c + 1) * 4],
                                     wsele[:, c * 4:(c + 1) * 4])
                red2 = vp.tile([128, 1], f32, tag="fred2", name=f"fred2_{c}")
                nc.vector.tensor_reduce(red2[:], scr2[:], axis=AX.X,
                                        op=ALU.add)
                nc.vector.tensor_add(facc[:], facc[:], red2[:])

            # ---- main loop: PE consumers of ACT outputs deferred one m ----
            pending = []
            for c in range(NC4):
                emit_start(c)
                for m in range(NM):
                    emit_kloop(c, m)
                    if pending:
                        pc, pm_ = pending.pop(0)
                        emit_leftover(pc, pm_)
                        if pm_ == NM - 1:
                            emit_oddsubs(pc)
                            emit_consume(pc, (0, 1, 2, 3))
                            emit_feat(pc)
                    pending.append((c, m))
            pc, pm_ = pending.pop(0)
            emit_leftover(pc, pm_)
            emit_consume(pc, (0, 2))   # subs 0/2 committed; DVE runs while
            emit_oddsubs(pc)           # ...PE finishes subs 1/3
            emit_consume(pc, (1, 3))
            emit_feat(pc)
            nc.sync.dma_start(d_feat, facc[:])
            if debug_out:
                nc.sync.dma_start(d_msed, mse_sb[:])
                nc.sync.dma_start(d_mbd, mb_sb[:])

    nc.compile()
    return nc


def _get_program(db_nonzero: bool, debug_out: bool = False):
    key = (bool(db_nonzero), bool(debug_out))
    if key not in _PROGRAM_CACHE:
        _PROGRAM_CACHE[key] = _build_program(*key)
    return _PROGRAM_CACHE[key]


# ----------------------------------------------------------------------------
# host side
# ----------------------------------------------------------------------------

def _host_scan_all(tg_all, sg_all, mask_f, gumbel):
    """Method-A sampling scan, all cores vectorized. Exact argmax semantics.
    Returns (wsel[B,S,E] f32, wsum f64, t_counts[E] f64, s_counts[E] f64)."""
    f32 = np.float32
    p = tg_all.astype(f32).copy()
    wsel = np.zeros((B, S, E), f32)
    BIG = f32(1e4)
    iota = np.arange(E, dtype=f32)
    for k in range(K):
        z = np.log(p) + gumbel[k]
        m = z.max(-1, keepdims=True)
        ge = (z >= m).astype(f32)
        t = iota + BIG - BIG * ge
        idxf = t.min(-1, keepdims=True)
        oh = (iota == idxf).astype(f32)
        po = p * oh
        w = po.sum(-1)
        mw = mask_f * w
        wsel += mw[..., None] * oh
        if k < K - 1:
            pn = p + (ALPHA - 1.0) * po
            p = pn / pn.sum(-1, keepdims=True)
    t_counts = wsel.astype(np.float64).sum(axis=(0, 1))
    wsum = float(t_counts.sum())
    # recompute s-side accumulation (needs per-step oh); cheap second pass
    p = tg_all.astype(f32).copy()
    s_counts = np.zeros(E, np.float64)
    for k in range(K):
        z = np.log(p) + gumbel[k]
        m = z.max(-1, keepdims=True)
        ge = (z >= m).astype(f32)
        t = iota + BIG - BIG * ge
        idxf = t.min(-1, keepdims=True)
        oh = (iota == idxf).astype(f32)
        po = p * oh
        sg_k = (sg_all * oh).sum(-1)
        s_counts += ((mask_f * sg_k)[..., None] * oh).astype(np.float64).sum(axis=(0, 1))
        if k < K - 1:
            pn = p + (ALPHA - 1.0) * po
            p = pn / pn.sum(-1, keepdims=True)
    return wsel, wsum, t_counts, s_counts


def _host_method_b(tg, sg, temp_c):
    """Per-core method-B partials: (tkl, ent)."""
    f32 = np.float32
    tg = tg.astype(f32)
    sg = sg.astype(f32)
    sgT = sg / f32(temp_c)
    ltg = np.log(tg)
    lsg = np.log(sg)
    ent = (sg * lsg).sum(dtype=f32)
    mb2 = sgT.max(-1, keepdims=True)
    ex = np.exp(sgT - mb2)
    se = ex.sum(-1, keepdims=True, dtype=f32)
    lse = np.log(se) + mb2
    sum_tg = tg.sum(-1, keepdims=True, dtype=f32)
    tkl = (tg * (ltg - sgT)).sum(dtype=f32) + (lse * sum_tg).sum(dtype=f32)
    return tkl, ent


def _prep_shared(inputs, db_nonzero):
    """Replicated (per-core identical) device arrays."""
    f32 = np.float32
    W_t = np.asarray(inputs["W_t"], f32)
    W_s = np.asarray(inputs["W_s"], f32)
    A_t = np.asarray(inputs["A_t"], f32)
    A_s = np.asarray(inputs["A_s"], f32)
    B_t = np.asarray(inputs["B_t"], f32)
    B_s = np.asarray(inputs["B_s"], f32)
    db = (np.asarray(inputs["b_s"], f32) - np.asarray(inputs["b_t"], f32))

    KT = NKX + (1 if db_nonzero else 0)

    # Wc layout [p, m, k*128 + c] = Wcat[m*128+c, k*128+p], fp8 pre-scaled.
    # k<16: W_s tiles; 16<=k<32: -W_t tiles; k=32 (db path): bias row on p=0.
    def w_tiles(W):
        return ((W * WSCALE).astype(FP8)
                .reshape(NM, 128, NM, 128).transpose(3, 0, 2, 1))  # [p,m,k,c]

    Wc = np.zeros((128, NM, KT, 128), FP8)
    Wc[:, :, 0:16, :] = w_tiles(W_s)
    Wc[:, :, 16:32, :] = w_tiles(-W_t)
    if db_nonzero:
        Wc[0, :, 32, :] = (db * WSCALE).astype(FP8).reshape(NM, 128)
    Wc = np.ascontiguousarray(Wc.reshape(128, NM, KT * 128))

    # Bc [p, mp, j, 256]: j = DoubleRow pair index (m = 2mp+j); 256 cols =
    # [BC_F*B_s_her | -BC_F*B_t_her] for that m's h-block.
    Bs_her = B_s.transpose(1, 0, 2).reshape(H, E * R)
    Bt_her = B_t.transpose(1, 0, 2).reshape(H, E * R)
    Bfull = np.concatenate([BC_F * Bs_her, -BC_F * Bt_her], axis=1)  # [H,256]
    Bc = np.ascontiguousarray(
        Bfull.reshape(8, 2, 128, 256).transpose(2, 0, 1, 3)
        .reshape(128, 8 * 512)).astype(FP8)

    # Gram pair strips [16, 2, 256] fp8, sharing the V accumulator's ALPHA_V
    G_ss = np.einsum("ehr,ehq->erq", B_s, B_s)
    G_st = np.einsum("ehr,ehq->erq", B_s, B_t)
    G_tt = np.einsum("ehr,ehq->erq", B_t, B_t)
    G_stT = G_st.transpose(0, 2, 1)

    def to_req(G):
        return G.transpose(1, 0, 2).reshape(R, E * R)

    Gs = np.concatenate([GC_F * to_req(G_ss), -GC_F * to_req(G_st)], axis=1)
    Gt = np.concatenate([-GC_F * to_req(G_stT), GC_F * to_req(G_tt)], axis=1)
    Gc = np.ascontiguousarray(
        np.stack([Gs, Gt], axis=1).reshape(16, 512)).astype(FP8)

    onesH = np.full((128, 1), 1.0 / (H * WSCALE * WSCALE), f32)

    shared = dict(Wc=Wc, Bc=Bc, Gc=Gc, onesH=onesH)
    mats = dict(A_sT=np.ascontiguousarray(A_s.T), A_tT=np.ascontiguousarray(A_t.T))
    return shared, mats, KT


def _prep_core(inputs, core, KT, wsel, mats):
    """Per-core device arrays."""
    f32 = np.float32
    sh = np.asarray(inputs["student_hidden_states"][core], f32)
    th = np.asarray(inputs["teacher_hidden_states"][core], f32)

    a_s = sh @ mats["A_sT"]                      # [S, R] f32
    a_t = th @ mats["A_tT"]
    acat = np.concatenate([a_s, a_t], axis=1) * f32(1.0 / ALPHA_V)  # [S, 32]
    acat = np.ascontiguousarray(
        acat.reshape(NCH, 128, 32).transpose(1, 0, 2).reshape(128, NCH * 32)
    ).astype(f32)
    aT = np.ascontiguousarray(
        np.stack([a_s.T, a_t.T], axis=1).reshape(16, 2 * S)).astype(FP8)

    # xc [p, chunk, k, s]: k<16 student, 16<=k<32 teacher, k=32 ones (db path)
    def x_tiles(x):
        return (x.T.astype(FP8).reshape(16, 128, NC4, 512)
                .transpose(1, 2, 0, 3))          # [p, c, k, s]

    xcv = np.zeros((128, NC4, KT, 512), FP8)
    xcv[:, :, 0:16, :] = x_tiles(sh)
    xcv[:, :, 16:32, :] = x_tiles(th)
    if KT > NKX:
        xcv[0, :, 32, :] = FP8(1.0)
    xcv = np.ascontiguousarray(xcv.reshape(128, NC4, KT * 512))

    wsel_dev = np.ascontiguousarray(
        wsel.reshape(NCH, 128, E).transpose(1, 0, 2).reshape(128, 128)).astype(f32)
    wsel_e = np.ascontiguousarray(wsel.sum(-1).reshape(NCH, 128).T).astype(f32)
    return dict(xc=xcv, wsel=wsel_dev, wsel_e=wsel_e, acat=acat, aT=aT)


def _combine(feat_parts, wsum, t_counts, s_counts, tkls, ents, temp_c):
    f32 = np.float32
    feat = np.sum(np.asarray(feat_parts, f32), dtype=f32)
    tc = np.asarray(t_counts, np.float64)
    sc = np.asarray(s_counts, np.float64)
    tkl = np.sum(np.asarray(tkls, f32), dtype=f32)
    ent = np.sum(np.asarray(ents, f32), dtype=f32)

    feat_loss = feat / max(wsum, 1e-8)
    t_avg = tc / tc.sum() + EPS
    s_avg = sc / sc.sum() + EPS
    t_avg = t_avg / t_avg.sum()
    s_avg = s_avg / s_avg.sum()
    coverage_kl = (t_avg * (np.log(t_avg) - np.log(s_avg))).sum() / E
    method_a_total = feat_loss + LAMBDA_COV * coverage_kl
    temp_kl = tkl / B
    entropy_loss = ent / (B * S)
    method_b_total = temp_kl + BETA_ENT * entropy_loss
    return np.array(
        [feat_loss, coverage_kl, method_a_total, temp_kl, entropy_loss,
         method_b_total, temp_c], f32)


def _host_all(inputs):
    """Host scan/method-B for all cores + per-core device input maps."""
    f32 = np.float32
    db_nonzero = bool(
        np.any(np.asarray(inputs["b_s"], f32) != np.asarray(inputs["b_t"], f32)))
    temp = float(np.asarray(inputs["temperature"], f32))
    temp_c = float(np.clip(temp, TEMP_LO, TEMP_HI))

    u = np.asarray(inputs["uniform_noise"], f32)
    gumbel = -np.log(-np.log(u * (1.0 - 2e-7) + 1e-7)).astype(f32)
    mask_f = np.asarray(inputs["attention_mask"], f32)
    tg_all = np.asarray(inputs["teacher_gates"], f32)
    sg_all = np.asarray(inputs["student_gates"], f32)

    shared, mats, KT = _prep_shared(inputs, db_nonzero)
    wsel_all, wsum, t_counts, s_counts = _host_scan_all(
        tg_all, sg_all, mask_f, gumbel)

    in_maps = []
    tkls, ents = [], []
    for c in range(B):
        tkl, ent = _host_method_b(tg_all[c], sg_all[c], temp_c)
        tkls.append(tkl)
        ents.append(ent)
        m = dict(shared)
        m.update(_prep_core(inputs, c, KT, wsel_all[c], mats))
        in_maps.append(m)

    return dict(in_maps=in_maps, db_nonzero=db_nonzero, temp_c=temp_c,
                wsum=wsum, t_counts=t_counts, s_counts=s_counts,
                tkls=tkls, ents=ents)


def kernel(**inputs) -> np.ndarray:
    host = _host_all(inputs)
    nc = _get_program(host["db_nonzero"])

    from concourse.bass_utils import run_bass_kernel_spmd

    res = run_bass_kernel_spmd(nc, host["in_maps"], core_ids=list(range(B)))
    feat_parts = [float(np.asarray(res.results[c]["feat"], np.float32).sum())
                  for c in range(B)]

    return _combine(feat_parts, host["wsum"], host["t_counts"],
                    host["s_counts"], host["tkls"], host["ents"],
                    host["temp_c"])


# revision 27
# speedup vs baseline: 1.0398x; 1.0398x over previous
"""Trainium2 Bass kernel for nn_ExpertDistillationLoss.

Strategy (data-parallel over batch, 8 cores, 1 batch element each):
  - Device (per core): the FLOP-heavy expert-MSE pipeline.
      d.T[h, s] = W_s.sh.T - W_t.th.T computed as one concatenated fp8
      DoubleRow GEMM (W stationary & SBUF-resident, loaded once; host
      pre-transposed layouts; f32 PSUM accumulation).
      mean_base via ACT square + per-tile ones-matmul PSUM accumulation.
      cross+quad terms fused into one PSUM accumulator V[s, 256] built from
      (a) fp8 DoubleRow P-matmuls of dT m-tile PAIRS against host-prescaled
          B_cat and
      (b) one fp8 DoubleRow Gram matmul per token tile (as/at paired),
      then a broadcasted DVE multiply/reduce against a_s/a_t.
      Device output per core: feat partial = sum wsel*mse (1 scalar).
  - Host: input sharding/layout, the K=3 MC sampling scan (gates-only, exact
    argmax semantics), method-B losses, and the final scalar combine.
"""

import numpy as np
import ml_dtypes

B, S, H, E, R, K = 8, 2048, 2048, 8, 16, 3
ALPHA = 0.5
LAMBDA_COV = 0.5
BETA_ENT = 0.1
TEMP_LO, TEMP_HI = 0.5, 1.5
SCALE_T = 2.0
SCALE_S = 2.0
EPS = 1e-8

NM = 16                # output h-tiles (128 rows each)
NKX = 32               # k-tiles: 16 student + 16 teacher
NC4 = 4                # 512-token chunks
NSUB = 4               # 128-token subchunks per chunk
NCH = 16               # 128-token chunks over S

BF16 = ml_dtypes.bfloat16
FP8 = ml_dtypes.float8_e4m3fn
WSCALE = 64.0          # W pre-scale so fp8 e4m3 stays in normal range
DCOPY = 0.25           # dT = DCOPY * pd = (WSCALE*DCOPY) * d = 16 d
ALPHA_V = 131072.0     # 2**17: common scale carried by the V accumulator
BC_F = ALPHA_V * 2.0 * SCALE_S / (H * WSCALE * DCOPY)   # = 16.0
GC_F = ALPHA_V * SCALE_S * SCALE_T / H                  # = 256.0

_PROGRAM_CACHE = {}


# ----------------------------------------------------------------------------
# device program
# ----------------------------------------------------------------------------

def _build_program(db_nonzero: bool, debug_out: bool = False):
    import concourse.bacc as bacc
    import concourse.tile as tile
    from concourse import mybir

    f32 = mybir.dt.float32
    bf16 = mybir.dt.bfloat16
    fp8 = mybir.dt.float8e4
    DR = mybir.MatmulPerfMode.DoubleRow
    ALU = mybir.AluOpType
    AX = mybir.AxisListType

    KT = NKX + (1 if db_nonzero else 0)   # extra k-tile carries the bias row
    WB = KT * 128                          # W cols per m-tile
    XB = KT * 512                          # x cols per 512-token chunk

    nc = bacc.Bacc("TRN2", target_bir_lowering=False, debug=False)

    # DRAM inputs (per-core shapes; layouts are host-prepared)
    d_xc = nc.dram_tensor("xc", [128, NC4, XB], fp8, kind="ExternalInput").ap()
    d_Wc = nc.dram_tensor("Wc", [128, NM, WB], fp8, kind="ExternalInput").ap()
    d_Bc = nc.dram_tensor("Bc", [128, 8 * 512], fp8, kind="ExternalInput").ap()
    d_Gc = nc.dram_tensor("Gc", [16, 512], fp8, kind="ExternalInput").ap()
    d_aT = nc.dram_tensor("aT", [16, 2 * S], fp8, kind="ExternalInput").ap()
    d_abw = nc.dram_tensor("abw", [128, NCH * 256], f32, kind="ExternalInput").ap()
    d_wsele = nc.dram_tensor("wsel_e", [128, 16], f32, kind="ExternalInput").ap()
    d_onesH = nc.dram_tensor("onesH", [128, 1], f32, kind="ExternalInput").ap()

    d_feat = nc.dram_tensor("feat", [128, 20], f32, kind="ExternalOutput").ap()
    if debug_out:
        d_mbd = nc.dram_tensor("mb_dbg", [128, 16], f32, kind="ExternalOutput").ap()

    with tile.TileContext(nc) as tc:
        with (
            tc.tile_pool(name="const", bufs=1) as cp,
            tc.tile_pool(name="dT", bufs=2) as dp,
            tc.tile_pool(name="sq", bufs=3) as qp,
            tc.tile_pool(name="vc", bufs=4) as vp,
            tc.tile_pool(name="pd", bufs=3, space="PSUM") as pd,
            tc.tile_pool(name="pv", bufs=4, space="PSUM") as pv,
            tc.tile_pool(name="pm", bufs=1, space="PSUM") as pm,
        ):
            # ---- SBUF tiles ----
            Gc_sb = cp.tile([16, 512], fp8, tag="Gc")
            aT_sb = cp.tile([16, 2 * S], fp8, tag="aT")
            Wc = cp.tile([128, NM * WB], fp8, tag="Wc")
            xc = cp.tile([128, NC4 * XB], fp8, tag="xc")
            Bc = cp.tile([128, 8 * 512], fp8, tag="Bc")
            abw_sb = cp.tile([128, NCH * 256], f32, tag="abw")
            wsele = cp.tile([128, 16], f32, tag="wsele")
            onesH = cp.tile([128, 1], f32, tag="onesH")
            mb_sb = cp.tile([128, 16], f32, tag="mb")
            fparts = cp.tile([128, 20], f32, tag="fparts")

            # ---- DMA emission order (HWDGE serializes at ~625ns/DMA and the
            # DMA bus at ~360B/ns shared; order = need order on the PE).
            # W must stream ahead of the PE's ~1.9us/m-tile cadence, so after
            # chunk 0's x data the W tiles go out back-to-back; later x chunks
            # ride behind the full W set.
            dma = nc.sync.dma_start
            dma(Gc_sb[:], d_Gc)
            dma(aT_sb[:], d_aT)
            dma(Wc[:, 0:256], d_Wc[:, 0, 0:256])          # m0 kp0
            dma(xc[:, 0:1024], d_xc[:, 0, 0:1024])        # c0 kp0
            dma(Wc[:, 256:WB], d_Wc[:, 0, 256:WB])        # m0 rest
            dma(xc[:, 1024:4096], d_xc[:, 0, 1024:4096])  # c0 kp1-3
            dma(xc[:, 4096:8192], d_xc[:, 0, 4096:8192])  # c0 kp4-7
            dma(Wc[:, WB:2 * WB], d_Wc[:, 1, :])
            dma(xc[:, 8192:12288], d_xc[:, 0, 8192:12288])
            dma(xc[:, 12288:XB], d_xc[:, 0, 12288:XB])
            dma(Wc[:, 2 * WB:3 * WB], d_Wc[:, 2, :])
            dma(Bc[:], d_Bc)
            dma(onesH[:], d_onesH)
            for m in range(3, 16):
                dma(Wc[:, m * WB:(m + 1) * WB], d_Wc[:, m, :])
            for q in range(4):                             # c1 in 4 pieces
                dma(xc[:, XB + q * 4096:XB + (q + 1) * 4096],
                    d_xc[:, 1, q * 4096:(q + 1) * 4096])
            dma(abw_sb[:], d_abw)
            dma(wsele[:], d_wsele)
            dma(xc[:, 2 * XB:3 * XB], d_xc[:, 2, :])
            dma(xc[:, 3 * XB:4 * XB], d_xc[:, 3, :])

            # ---- views ----
            W4 = Wc[:].rearrange("p (m k c) -> p m k c", m=NM, k=KT)
            x4 = xc[:].rearrange("p (n k s) -> p n k s", n=NC4, k=KT)
            aT2 = aT_sb[:].rearrange("p (j s) -> p j s", j=2)
            Gc2 = Gc_sb[:].rearrange("p (j g) -> p j g", j=2)
            Bc3 = Bc[:].rearrange("p (mp j g) -> p mp j g", mp=8, j=2)

            # PSUM rule (probe-verified): a bank holds ONE open accumulation
            # group; a start=True wipes any other OPEN group's partials in
            # that bank (committed/stopped results survive). So: V banks run
            # one sub's full chain at a time (subs 0/2 during the m-loop,
            # subs 1/3 afterwards from the dT cache), and mean_base uses
            # per-(m,sub) single-shot matmuls + a DVE reduction over m.
            V_of = {}    # c -> [2 psum tiles of [128, 512] (2 subs each)]
            mb_of = {}   # c -> [128, 64] psum tile of per-(m,sub) sums
            sq_of = {}   # (c, m) -> sq tile
            dT_of = {}   # c -> [128, 8*1024] fp8 dT cache (mp, j, 512)

            def Vap(c, sub):
                t = V_of[c][sub // 2]
                return t[:, (sub % 2) * 256:(sub % 2) * 256 + 256]

            def emit_u(c, subs):
                for sub in subs:
                    t0 = c * 512 + sub * 128
                    nc.tensor.matmul(Vap(c, sub), aT2[:, :, t0:t0 + 128],
                                     Gc2, start=True, stop=False,
                                     perf_mode=DR)

            def emit_pmm(c, mp, subs):
                dT3 = dT_of[c][:].rearrange("p (mp j s) -> p mp j s",
                                            mp=8, j=2)
                for sub in subs:
                    nc.tensor.matmul(
                        Vap(c, sub),
                        dT3[:, mp, :, sub * 128:sub * 128 + 128],
                        Bc3[:, mp], start=False, stop=(mp == 7),
                        perf_mode=DR)

            def emit_start(c):
                V_of[c] = [pv.tile([128, 512], f32, tag="V", name=f"V_{c}_{i}")
                           for i in range(2)]
                mb_of[c] = pm.tile([128, 64], f32, tag="mbp", name=f"mb_{c}")
                dT_of[c] = dp.tile([128, 8 * 1024], fp8, tag="dT",
                                   name=f"dTall_{c}")
                emit_u(c, (0, 2))

            def emit_kloop(c, m):
                pdt = pd.tile([128, 512], f32, tag="pd", name=f"pd_{c}_{m}")
                for kp in range(NKX // 2):
                    nc.tensor.matmul(
                        pdt[:], W4[:, m, 2 * kp:2 * kp + 2, :],
                        x4[:, c, 2 * kp:2 * kp + 2, :],
                        start=(kp == 0),
                        stop=(kp == NKX // 2 - 1 and KT == NKX),
                        perf_mode=DR)
                if KT > NKX:
                    # bias tail tile: plain (non-DoubleRow) fp8 matmul
                    nc.tensor.matmul(pdt[:], W4[:, m, NKX:NKX + 1, :],
                                     x4[:, c, NKX:NKX + 1, :],
                                     start=False, stop=True)
                # ACT ops run async while PE streams the next k-loop
                nc.scalar.mul(dT_of[c][:, m * 512:m * 512 + 512],
                              pdt[:], DCOPY)
                sq = qp.tile([128, 512], f32, tag="sq", name=f"sq_{c}_{m}")
                nc.scalar.square(sq[:], pdt[:])
                sq_of[(c, m)] = sq

            def emit_leftover(c, m):
                # P-matmuls first: they gate the consume/feat tail chain,
                # and the dT copy lands on ACT before the square does
                if m % 2 == 1:
                    emit_pmm(c, m // 2, (0, 2))
                # mean_base: per-(m,sub) single-shot ones-matmuls
                sq = sq_of.pop((c, m))
                mbp = mb_of[c]
                for sub in range(NSUB):
                    col = m * NSUB + sub
                    nc.tensor.matmul(mbp[:, col:col + 1],
                                     sq[:, sub * 128:sub * 128 + 128],
                                     onesH[:], start=True, stop=True)

            def emit_oddsubs(c):
                # subs 1/3 full chains after subs 0/2 committed (stop at mp7);
                # sub 1 completes first so its consume can overlap sub 3's
                for sub in (1, 3):
                    emit_u(c, (sub,))
                    for mp in range(8):
                        emit_pmm(c, mp, (sub,))

            def emit_consume(c, subs):
                # abw carries ab * wsel (host-fused), so each sub reduces
                # straight to its feat partial column
                for sub in subs:
                    ch = c * NSUB + sub
                    va = Vap(c, sub)
                    prod = vp.tile([128, 256], bf16, tag="prod",
                                   name=f"prod_{ch}")
                    nc.vector.tensor_tensor(
                        prod[:], va, abw_sb[:, ch * 256:(ch + 1) * 256],
                        ALU.mult)
                    nc.vector.tensor_reduce(
                        fparts[:, ch:ch + 1], prod[:], axis=AX.X, op=ALU.add)

            def emit_feat(c):
                # mean_base partial for this chunk -> fparts col 16+c
                mbp = mb_of.pop(c)
                nc.vector.tensor_reduce(
                    mb_sb[:, c * NSUB:(c + 1) * NSUB],
                    mbp[:].rearrange("p (m s) -> p s m", m=NM),
                    axis=AX.X, op=ALU.add)
                scr2 = vp.tile([128, 4], f32, tag="scr2", name=f"scr2_{c}")
                nc.vector.tensor_mul(scr2[:], mb_sb[:, c * 4:(c + 1) * 4],
                                     wsele[:, c * 4:(c + 1) * 4])
                nc.vector.tensor_reduce(fparts[:, 16 + c:17 + c], scr2[:],
                                        axis=AX.X, op=ALU.add)

            # ---- main loop: PE consumers of ACT outputs deferred one m ----
            pending = []
            for c in range(NC4):
                emit_start(c)
                for m in range(NM):
                    emit_kloop(c, m)
                    if pending:
                        pc, pm_ = pending.pop(0)
                        emit_leftover(pc, pm_)
                        if pm_ == NM - 1:
                            emit_oddsubs(pc)
                            emit_feat(pc)
                            emit_consume(pc, (0, 1, 2, 3))
                            V_of.pop(pc)
                            dT_of.pop(pc)
                    pending.append((c, m))
            pc, pm_ = pending.pop(0)
            emit_leftover(pc, pm_)
            emit_consume(pc, (0, 2))   # DVE runs while PE finishes subs 1/3
            emit_oddsubs(pc)
            emit_consume(pc, (1, 3))
            emit_feat(pc)
            V_of.pop(pc)
            dT_of.pop(pc)
            nc.sync.dma_start(d_feat, fparts[:])
            if debug_out:
                nc.sync.dma_start(d_mbd, mb_sb[:])

    nc.compile()
    return nc


def _get_program(db_nonzero: bool, debug_out: bool = False):
    key = (bool(db_nonzero), bool(debug_out))
    if key not in _PROGRAM_CACHE:
        _PROGRAM_CACHE[key] = _build_program(*key)
    return _PROGRAM_CACHE[key]


# ----------------------------------------------------------------------------
# host side
# ----------------------------------------------------------------------------

def _host_scan_all(tg_all, sg_all, mask_f, gumbel):
    """Method-A sampling scan, all cores vectorized. Exact argmax semantics.
    Returns (wsel[B,S,E] f32, wsum f64, t_counts[E] f64, s_counts[E] f64)."""
    f32 = np.float32
    p = tg_all.astype(f32).copy()
    wsel = np.zeros((B, S, E), f32)
    BIG = f32(1e4)
    iota = np.arange(E, dtype=f32)
    for k in range(K):
        z = np.log(p) + gumbel[k]
        m = z.max(-1, keepdims=True)
        ge = (z >= m).astype(f32)
        t = iota + BIG - BIG * ge
        idxf = t.min(-1, keepdims=True)
        oh = (iota == idxf).astype(f32)
        po = p * oh
        w = po.sum(-1)
        mw = mask_f * w
        wsel += mw[..., None] * oh
        if k < K - 1:
            pn = p + (ALPHA - 1.0) * po
            p = pn / pn.sum(-1, keepdims=True)
    t_counts = wsel.astype(np.float64).sum(axis=(0, 1))
    wsum = float(t_counts.sum())
    # recompute s-side accumulation (needs per-step oh); cheap second pass
    p = tg_all.astype(f32).copy()
    s_counts = np.zeros(E, np.float64)
    for k in range(K):
        z = np.log(p) + gumbel[k]
        m = z.max(-1, keepdims=True)
        ge = (z >= m).astype(f32)
        t = iota + BIG - BIG * ge
        idxf = t.min(-1, keepdims=True)
        oh = (iota == idxf).astype(f32)
        po = p * oh
        sg_k = (sg_all * oh).sum(-1)
        s_counts += ((mask_f * sg_k)[..., None] * oh).astype(np.float64).sum(axis=(0, 1))
        if k < K - 1:
            pn = p + (ALPHA - 1.0) * po
            p = pn / pn.sum(-1, keepdims=True)
    return wsel, wsum, t_counts, s_counts


def _host_method_b(tg, sg, temp_c):
    """Per-core method-B partials: (tkl, ent)."""
    f32 = np.float32
    tg = tg.astype(f32)
    sg = sg.astype(f32)
    sgT = sg / f32(temp_c)
    ltg = np.log(tg)
    lsg = np.log(sg)
    ent = (sg * lsg).sum(dtype=f32)
    mb2 = sgT.max(-1, keepdims=True)
    ex = np.exp(sgT - mb2)
    se = ex.sum(-1, keepdims=True, dtype=f32)
    lse = np.log(se) + mb2
    sum_tg = tg.sum(-1, keepdims=True, dtype=f32)
    tkl = (tg * (ltg - sgT)).sum(dtype=f32) + (lse * sum_tg).sum(dtype=f32)
    return tkl, ent


def _prep_shared(inputs, db_nonzero):
    """Replicated (per-core identical) device arrays."""
    f32 = np.float32
    W_t = np.asarray(inputs["W_t"], f32)
    W_s = np.asarray(inputs["W_s"], f32)
    A_t = np.asarray(inputs["A_t"], f32)
    A_s = np.asarray(inputs["A_s"], f32)
    B_t = np.asarray(inputs["B_t"], f32)
    B_s = np.asarray(inputs["B_s"], f32)
    db = (np.asarray(inputs["b_s"], f32) - np.asarray(inputs["b_t"], f32))

    KT = NKX + (1 if db_nonzero else 0)

    # Wc layout [p, m, k*128 + c] = Wcat[m*128+c, k*128+p], fp8 pre-scaled.
    # k<16: W_s tiles; 16<=k<32: -W_t tiles; k=32 (db path): bias row on p=0.
    def w_tiles(W):
        return ((W * WSCALE).astype(FP8)
                .reshape(NM, 128, NM, 128).transpose(3, 0, 2, 1))  # [p,m,k,c]

    Wc = np.zeros((128, NM, KT, 128), FP8)
    Wc[:, :, 0:16, :] = w_tiles(W_s)
    Wc[:, :, 16:32, :] = w_tiles(-W_t)
    if db_nonzero:
        Wc[0, :, 32, :] = (db * WSCALE).astype(FP8).reshape(NM, 128)
    Wc = np.ascontiguousarray(Wc.reshape(128, NM, KT * 128))

    # Bc [p, mp, j, 256]: j = DoubleRow pair index (m = 2mp+j); 256 cols =
    # [BC_F*B_s_her | -BC_F*B_t_her] for that m's h-block.
    Bs_her = B_s.transpose(1, 0, 2).reshape(H, E * R)
    Bt_her = B_t.transpose(1, 0, 2).reshape(H, E * R)
    Bfull = np.concatenate([BC_F * Bs_her, -BC_F * Bt_her], axis=1)  # [H,256]
    Bc = np.ascontiguousarray(
        Bfull.reshape(8, 2, 128, 256).transpose(2, 0, 1, 3)
        .reshape(128, 8 * 512)).astype(FP8)

    # Gram pair strips [16, 2, 256] fp8, sharing the V accumulator's ALPHA_V
    G_ss = np.einsum("ehr,ehq->erq", B_s, B_s)
    G_st = np.einsum("ehr,ehq->erq", B_s, B_t)
    G_tt = np.einsum("ehr,ehq->erq", B_t, B_t)
    G_stT = G_st.transpose(0, 2, 1)

    def to_req(G):
        return G.transpose(1, 0, 2).reshape(R, E * R)

    Gs = np.concatenate([GC_F * to_req(G_ss), -GC_F * to_req(G_st)], axis=1)
    Gt = np.concatenate([-GC_F * to_req(G_stT), GC_F * to_req(G_tt)], axis=1)
    Gc = np.ascontiguousarray(
        np.stack([Gs, Gt], axis=1).reshape(16, 512)).astype(FP8)

    onesH = np.full((128, 1), 1.0 / (H * WSCALE * WSCALE), f32)

    shared = dict(Wc=Wc, Bc=Bc, Gc=Gc, onesH=onesH)
    mats = dict(A_sT=np.ascontiguousarray(A_s.T), A_tT=np.ascontiguousarray(A_t.T))
    return shared, mats, KT


def _prep_core(inputs, core, KT, wsel, mats):
    """Per-core device arrays."""
    f32 = np.float32
    sh = np.asarray(inputs["student_hidden_states"][core], f32)
    th = np.asarray(inputs["teacher_hidden_states"][core], f32)

    a_s = sh @ mats["A_sT"]                      # [S, R] f32
    a_t = th @ mats["A_tT"]
    # abw[t, j, e, r] = (a/ALPHA_V)[t, j, r] * wsel[t, e]: the V-consume
    # multiplier with the feat weighting folded in
    ab = np.stack([a_s, a_t], axis=1) * f32(1.0 / ALPHA_V)   # [S, 2, R]
    abw = ab[:, :, None, :] * wsel.astype(f32)[:, None, :, None]  # [S,2,E,R]
    abw = np.ascontiguousarray(
        abw.reshape(NCH, 128, 256).transpose(1, 0, 2).reshape(128, NCH * 256)
    ).astype(f32)
    aT = np.ascontiguousarray(
        np.stack([a_s.T, a_t.T], axis=1).reshape(16, 2 * S)).astype(FP8)

    # xc [p, chunk, k, s]: k<16 student, 16<=k<32 teacher, k=32 ones (db path)
    def x_tiles(x):
        return (x.T.astype(FP8).reshape(16, 128, NC4, 512)
                .transpose(1, 2, 0, 3))          # [p, c, k, s]

    xcv = np.zeros((128, NC4, KT, 512), FP8)
    xcv[:, :, 0:16, :] = x_tiles(sh)
    xcv[:, :, 16:32, :] = x_tiles(th)
    if KT > NKX:
        xcv[0, :, 32, :] = FP8(1.0)
    xcv = np.ascontiguousarray(xcv.reshape(128, NC4, KT * 512))

    wsel_e = np.ascontiguousarray(wsel.sum(-1).reshape(NCH, 128).T).astype(f32)
    return dict(xc=xcv, wsel_e=wsel_e, abw=abw, aT=aT)


def _combine(feat_parts, wsum, t_counts, s_counts, tkls, ents, temp_c):
    f32 = np.float32
    feat = np.sum(np.asarray(feat_parts, f32), dtype=f32)
    tc = np.asarray(t_counts, np.float64)
    sc = np.asarray(s_counts, np.float64)
    tkl = np.sum(np.asarray(tkls, f32), dtype=f32)
    ent = np.sum(np.asarray(ents, f32), dtype=f32)

    feat_loss = feat / max(wsum, 1e-8)
    t_avg = tc / tc.sum() + EPS
    s_avg = sc / sc.sum() + EPS
    t_avg = t_avg / t_avg.sum()
    s_avg = s_avg / s_avg.sum()
    coverage_kl = (t_avg * (np.log(t_avg) - np.log(s_avg))).sum() / E
    method_a_total = feat_loss + LAMBDA_COV * coverage_kl
    temp_kl = tkl / B
    entropy_loss = ent / (B * S)
    method_b_total = temp_kl + BETA_ENT * entropy_loss
    return np.array(
        [feat_loss, coverage_kl, method_a_total, temp_kl, entropy_loss,
         method_b_total, temp_c], f32)


def _host_all(inputs):
    """Host scan/method-B for all cores + per-core device input maps."""
    f32 = np.float32
    db_nonzero = bool(
        np.any(np.asarray(inputs["b_s"], f32) != np.asarray(inputs["b_t"], f32)))
    temp = float(np.asarray(inputs["temperature"], f32))
    temp_c = float(np.clip(temp, TEMP_LO, TEMP_HI))

    u = np.asarray(inputs["uniform_noise"], f32)
    gumbel = -np.log(-np.log(u * (1.0 - 2e-7) + 1e-7)).astype(f32)
    mask_f = np.asarray(inputs["attention_mask"], f32)
    tg_all = np.asarray(inputs["teacher_gates"], f32)
    sg_all = np.asarray(inputs["student_gates"], f32)

    shared, mats, KT = _prep_shared(inputs, db_nonzero)
    wsel_all, wsum, t_counts, s_counts = _host_scan_all(
        tg_all, sg_all, mask_f, gumbel)

    in_maps = []
    tkls, ents = [], []
    for c in range(B):
        tkl, ent = _host_method_b(tg_all[c], sg_all[c], temp_c)
        tkls.append(tkl)
        ents.append(ent)
        m = dict(shared)
        m.update(_prep_core(inputs, c, KT, wsel_all[c], mats))
        in_maps.append(m)

    return dict(in_maps=in_maps, db_nonzero=db_nonzero, temp_c=temp_c,
                wsum=wsum, t_counts=t_counts, s_counts=s_counts,
                tkls=tkls, ents=ents)


def kernel(**inputs) -> np.ndarray:
    host = _host_all(inputs)
    nc = _get_program(host["db_nonzero"])

    from concourse.bass_utils import run_bass_kernel_spmd

    res = run_bass_kernel_spmd(nc, host["in_maps"], core_ids=list(range(B)))
    feat_parts = [float(np.asarray(res.results[c]["feat"], np.float32).sum())
                  for c in range(B)]

    return _combine(feat_parts, host["wsum"], host["t_counts"],
                    host["s_counts"], host["tkls"], host["ents"],
                    host["temp_c"])
